# revision 38
# baseline (speedup 1.0000x reference)
"""Trainium2 Bass kernel for nn_Attn_25409026523783.

Dense causal multi-head attention block (B=64, S=256, D=2048, H=16, HD=128):
    qkv = x @ w_qkv.T ; causal softmax attention per head ; out = ctx @ w_o.T

Strategy (fused bf16, zero DRAM spill; ~1.02 ms/core, PE ~96% busy at the
1 cycle/row bf16 stream roofline):
  - Batch-shard across the 8 NeuronCores (8 batches / 2048 tokens per core).
    No collectives: host scatters inputs, concatenates per-core outputs.
  - All operands cast to bf16 on the host and pre-tiled so every DMA is a
    contiguous block with the contraction dim on partitions. Matmuls run
    bf16 x bf16 -> fp32 PSUM (1 cycle/row at N>=256, measured same speed as
    fp32r, but half the DMA/SBUF footprint and cheaper LDWEIGHTS).
  - Head-major loop with x^T resident in SBUF (64 KiB/partition bf16):
    per head, project Q^T/K^T (feature-major) and V (token-major, produced
    per 4-head group), run causal attention for all 8 local batches, and
    write ctx^T into a resident bf16 buffer. Q/K/V never touch DRAM, and
    the PE interleaves head h's attention with head h+1's projections.
  - Attention per (batch, head) in transposed layout S^T[k, q] with causal
    skip (k-tile 1 only computed for queries >= 128): one exp (ACT, scale
    folded), one mask multiply (DVE, bf16 2x), denominators via ones-column
    matmul, reciprocal via DVE reciprocal_approx_fast, 1/den broadcast on
    the otherwise-idle GpSimd (partition_broadcast), ctx^T from V x A^T.
  - Loop-order notes from HW traces: accumulation chains must stay dt-inner
    (one PSUM bank per chain — bank-cycling between consecutive matmuls
    slows the PE ~18%), and gpsimd.partition_all_reduce for the denominators
    is a net loss (~2.8 us/op stalls the pipeline).
  - After the head loop x^T is freed; w_o^T streams in per-512-column
    chunks and the output projection accumulates ctx^T tiles over heads.
"""

import os
import sys

import numpy as np
import ml_dtypes

for _p in ("/opt/trn_rl_repo", "/root/.axon_site/_ro/trn_rl_repo"):
    if os.path.isdir(_p) and _p not in sys.path:
        sys.path.insert(0, _p)

import concourse.bass as bass  # noqa: E402,F401
import concourse.mybir as mybir  # noqa: E402
import concourse.tile as tile  # noqa: E402
from concourse import bacc  # noqa: E402
from concourse.bass_utils import run_bass_kernel_spmd  # noqa: E402


def _ensure_ntff_hook():
    """Some images ship an ``antenv`` without ``axon_hooks``; bass_utils then
    crashes on import when tracing is requested. Provide a no-op-compatible
    module (and register the real ctypes hook when available)."""
    try:
        from antenv import axon_hooks  # noqa: F401
        return
    except ImportError:
        pass
    import types

    mod = types.ModuleType("antenv.axon_hooks")
    mod._hook = None
    mod.set_axon_ntff_profile_hook = lambda h: setattr(mod, "_hook", h)
    mod.get_axon_ntff_profile_hook = lambda: mod._hook
    sys.modules["antenv.axon_hooks"] = mod
    try:
        import antenv

        antenv.axon_hooks = mod
        from trn_agent_boot.trn_boot import _ntff_profile_via_ctypes

        hook = _ntff_profile_via_ctypes("/opt/axon/libaxon_pjrt.so")
        if hook is not None:
            mod._hook = hook
    except Exception:
        pass


_ensure_ntff_hook()

F32 = mybir.dt.float32
F32R = mybir.dt.float32r
BF16 = mybir.dt.bfloat16
EXP = mybir.ActivationFunctionType.Exp

# Problem constants (per spec, hardcoded)
B, S, D, H = 64, 256, 2048, 16
HD = D // H  # 128
N_CORES = 8
NB = B // N_CORES           # 8 batches per core
T = NB * S                  # 2048 tokens per core
P = 128
DT = D // P                 # 16 d-tiles
SCALE = float(HD) ** -0.5
G = 4                       # heads per V-production group

_CACHE = {}


def _build():
    nc = bacc.Bacc("TRN2", target_bir_lowering=False, debug=False,
                   enable_asserts=False)

    xt_t = nc.dram_tensor("xt_t", [P, DT, T], BF16, kind="ExternalInput").ap()
    # per-head Q|K weight stripes: [h, p(d), dt, 256] (q cols 0:128, k 128:256)
    wqk_t = nc.dram_tensor("wqk_t", [H, P, DT, 256], BF16,
                           kind="ExternalInput").ap()
    # V weight stripes per 4-head group: [g, p(d), dt, 512]
    wv_t = nc.dram_tensor("wv_t", [G, P, DT, 512], BF16,
                          kind="ExternalInput").ap()
    # w_o^T stripes per 512-col output chunk: [ec, p(d'), h, 512]
    wo_t = nc.dram_tensor("wo_t", [D // 512, P, H, 512], BF16,
                          kind="ExternalInput").ap()
    # mask2 [128, 384]: [tri(k<=q) | ones | tri] (causal-skip layout)
    mask2 = nc.dram_tensor("mask2", [P, 3 * P], BF16,
                           kind="ExternalInput").ap()
    out = nc.dram_tensor("out", [T, D], F32, kind="ExternalOutput").ap()

    with tile.TileContext(nc) as tc:
        with tc.tile_pool(name="const", bufs=1) as c_pool, \
             tc.tile_pool(name="ctx", bufs=1) as ctx_pool:

            m2 = c_pool.tile([P, 3 * P], BF16)
            nc.sync.dma_start(out=m2[:], in_=mask2)
            ones_col = m2[:, 2 * P - 1:2 * P]   # all-ones [128, 1] bf16
            ones_row = m2[0:1, P:2 * P]         # all-ones [1, 128] bf16

            # resident ctx^T accumulator: [128 d', 16 h, 2048 t] bf16
            ctx_b = ctx_pool.tile([P, H, T], BF16, name="ctx_b")

            # ---------------- head loop: QKV + attention ----------------
            with tc.tile_pool(name="xt", bufs=1) as xt_pool, \
                 tc.tile_pool(name="wqk", bufs=2) as wqk_pool, \
                 tc.tile_pool(name="wv", bufs=1) as wv_pool, \
                 tc.tile_pool(name="qk", bufs=2) as qk_pool, \
                 tc.tile_pool(name="vg", bufs=1) as v_pool, \
                 tc.tile_pool(name="at", bufs=4) as a_pool, \
                 tc.tile_pool(name="rcp", bufs=3) as r_pool, \
                 tc.tile_pool(name="ps_qkv", bufs=3, space="PSUM") as qkv_ps, \
                 tc.tile_pool(name="ps_s", bufs=2, space="PSUM") as s_ps, \
                 tc.tile_pool(name="ps_db", bufs=1, space="PSUM") as db_ps, \
                 tc.tile_pool(name="ps_c", bufs=2, space="PSUM") as c_ps:

                # head-0 weights first so the PE can start a few us in; the
                # 8.4 MiB x^T load streams behind them. Split q|k halves so
                # the first (q) chains only wait on the 0.5 MiB q-half.
                wqk0 = wqk_pool.tile([P, DT, 256], BF16, tag="wqk")
                nc.sync.dma_start(out=wqk0[:, :, 0:P], in_=wqk_t[0][:, :, 0:P])
                nc.sync.dma_start(out=wqk0[:, :, P:2 * P],
                                  in_=wqk_t[0][:, :, P:2 * P])

                xt = xt_pool.tile([P, DT, T], BF16)
                for dt_ in range(DT):
                    nc.sync.dma_start(out=xt[:, dt_, :], in_=xt_t[:, dt_, :])

                wv0 = wv_pool.tile([P, DT, 512], BF16, tag="wv")
                nc.sync.dma_start(out=wv0[:], in_=wv_t[0])

                copy_i = 0
                vg = None
                for h in range(H):
                    hh = h % G
                    # -- weights for this head (and V group) --
                    if h == 0:
                        wqk = wqk0
                    else:
                        wqk = wqk_pool.tile([P, DT, 256], BF16, tag="wqk")
                        nc.sync.dma_start(out=wqk[:], in_=wqk_t[h])
                    if hh == 0:
                        vg = v_pool.tile([P, T // P, 512], BF16, tag="vg")
                        if h == 0:
                            wv = wv0
                        else:
                            wv = wv_pool.tile([P, DT, 512], BF16, tag="wv")
                            nc.sync.dma_start(out=wv[:], in_=wv_t[h // G])

                    # -- Q^T / K^T projection: [d', t] feature-major.
                    # dt-inner: each accumulation chain stays in one PSUM
                    # bank (bank-cycling between consecutive MMs measurably
                    # slows the PE).
                    qk = qk_pool.tile([P, 2, T], BF16, tag="qk")
                    for half in range(2):
                        for tch in range(T // 512):
                            ps = qkv_ps.tile([P, 512], F32, tag="qps",
                                             name=f"qps{half}{tch}")
                            for dt_ in range(DT):
                                nc.tensor.matmul(
                                    ps[:],
                                    wqk[:, dt_, half * P:(half + 1) * P],
                                    xt[:, dt_, tch * 512:(tch + 1) * 512],
                                    start=(dt_ == 0), stop=(dt_ == DT - 1),
                                )
                            if copy_i % 2 == 0:
                                nc.vector.tensor_copy(
                                    qk[:, half, tch * 512:(tch + 1) * 512],
                                    ps[:])
                            else:
                                nc.scalar.copy(
                                    qk[:, half, tch * 512:(tch + 1) * 512],
                                    ps[:])
                            copy_i += 1

                    # -- V projection for the 4-head group: [t, f] --
                    if hh == 0:
                        for tt in range(T // P):
                            ps = qkv_ps.tile([P, 512], F32, tag="qps",
                                             name=f"vps{tt}")
                            for dt_ in range(DT):
                                nc.tensor.matmul(
                                    ps[:],
                                    xt[:, dt_, tt * P:(tt + 1) * P],
                                    wv[:, dt_, :],
                                    start=(dt_ == 0), stop=(dt_ == DT - 1),
                                )
                            if copy_i % 2 == 0:
                                nc.vector.tensor_copy(vg[:, tt, :], ps[:])
                            else:
                                nc.scalar.copy(vg[:, tt, :], ps[:])
                            copy_i += 1

                    # -- causal attention for the 8 local batches.
                    # k-tile 1 only attends to queries q>=128 (causal skip):
                    # a_t columns [0:256] are k-tile 0 x all q, [256:384]
                    # are k-tile 1 x q in [128, 256).
                    for b in range(NB):
                        t0 = b * S
                        ps_s = s_ps.tile([P, S + P], F32)
                        nc.tensor.matmul(
                            ps_s[:, 0:S], qk[:, 1, t0:t0 + P],
                            qk[:, 0, t0:t0 + S], start=True, stop=True)
                        nc.tensor.matmul(
                            ps_s[:, S:S + P], qk[:, 1, t0 + P:t0 + S],
                            qk[:, 0, t0 + P:t0 + S], start=True, stop=True)
                        # A^T = exp(scale * S^T) * causal-mask
                        a_t = a_pool.tile([P, S + P], BF16, tag="at")
                        nc.scalar.activation(a_t[:], ps_s[:], EXP, scale=SCALE)
                        nc.vector.tensor_mul(a_t[:], a_t[:], m2[:])
                        # denominators [1, 256] via ones-column matmul
                        ps_db = db_ps.tile([1, S], F32)
                        nc.tensor.matmul(ps_db[0:1, 0:S], ones_col,
                                         a_t[:, 0:S],
                                         start=True, stop=False)
                        nc.tensor.matmul(ps_db[0:1, P:S], ones_col,
                                         a_t[:, S:S + P],
                                         start=False, stop=True,
                                         skip_group_check=True)
                        rcp = r_pool.tile([1, S], F32, tag="rcp")
                        nc.vector.reciprocal_approx_fast(
                            rcp[:], ps_db[0:1, 0:S])
                        # broadcast 1/denom across partitions on GpSimd
                        rb = r_pool.tile([P, S], F32, tag="rb")
                        nc.gpsimd.partition_broadcast(rb[:], rcp[:],
                                                      channels=P)
                        # ctx^T accumulated over the two k-tiles
                        ps_c = c_ps.tile([P, S], F32)
                        nc.tensor.matmul(
                            ps_c[:], vg[:, 2 * b, hh * P:(hh + 1) * P],
                            a_t[:, 0:S], start=True, stop=False)
                        nc.tensor.matmul(
                            ps_c[:, P:S], vg[:, 2 * b + 1, hh * P:(hh + 1) * P],
                            a_t[:, S:S + P], start=False, stop=True,
                            skip_group_check=True)
                        nc.vector.tensor_mul(ctx_b[:, h, t0:t0 + S],
                                             ps_c[:], rb[:])

            # ---------------- output projection ----------------
            with tc.tile_pool(name="wo", bufs=2) as wo_pool, \
                 tc.tile_pool(name="p3out", bufs=4) as o3_pool, \
                 tc.tile_pool(name="ps_o", bufs=2, space="PSUM") as o_ps:
                copy_i = 0
                for ec in range(D // 512):
                    wo = wo_pool.tile([P, H, 512], BF16, tag="wo")
                    nc.sync.dma_start(out=wo[:], in_=wo_t[ec])
                    for tt in range(T // P):
                        ps_o = o_ps.tile([P, 512], F32)
                        for h in range(H):
                            nc.tensor.matmul(
                                ps_o[:],
                                ctx_b[:, h, tt * P:(tt + 1) * P],
                                wo[:, h, :],
                                start=(h == 0), stop=(h == H - 1),
                            )
                        o_t = o3_pool.tile([P, 512], F32, tag="o3")
                        if copy_i % 2 == 0:
                            nc.vector.tensor_copy(o_t[:], ps_o[:])
                        else:
                            nc.scalar.copy(o_t[:], ps_o[:])
                        copy_i += 1
                        nc.sync.dma_start(
                            out=out[tt * P:(tt + 1) * P,
                                    ec * 512:(ec + 1) * 512],
                            in_=o_t[:],
                        )

    nc.compile()
    return nc


def get_nc():
    if "nc" not in _CACHE:
        _CACHE["nc"] = _build()
    return _CACHE["nc"]


def make_in_maps(x, w_qkv, w_o):
    x = np.ascontiguousarray(np.asarray(x, dtype=np.float32))
    w_qkv = np.asarray(w_qkv, dtype=np.float32)
    w_o = np.asarray(w_o, dtype=np.float32)
    bf = ml_dtypes.bfloat16
    # wqk_t [H, P, DT, 256]: [h,p,dt,j<128] = w_qkv[h*128+j, dt*128+p]
    wq = w_qkv[0:D].reshape(H, HD, DT, P).transpose(0, 3, 2, 1)
    wk = w_qkv[D:2 * D].reshape(H, HD, DT, P).transpose(0, 3, 2, 1)
    wqk = np.ascontiguousarray(
        np.concatenate([wq, wk], axis=3)).astype(bf)
    # wv_t [G, P, DT, 512]: [g,p,dt,j] = w_qkv[2D + g*512 + j, dt*128+p]
    wv = np.ascontiguousarray(
        w_qkv[2 * D:].reshape(G, 512, DT, P).transpose(0, 3, 2, 1)).astype(bf)
    # wo_t [EC, P, H, 512]: [ec,p,h,j] = w_o[ec*512+j, h*128+p]
    wo = np.ascontiguousarray(
        w_o.reshape(D // 512, 512, H, HD).transpose(0, 3, 2, 1)).astype(bf)
    # causal mask blocks: [tri(k<=q) | ones | tri]
    tri = np.triu(np.ones((P, P), dtype=np.float32))
    mask2 = np.concatenate(
        [tri, np.ones((P, P), np.float32), tri], axis=1).astype(bf)
    in_maps = []
    for c in range(N_CORES):
        xs = x[c * NB:(c + 1) * NB].reshape(T, D)
        xt = np.ascontiguousarray(
            xs.reshape(T, DT, P).transpose(2, 1, 0)).astype(bf)
        in_maps.append({"xt_t": xt, "wqk_t": wqk, "wv_t": wv, "wo_t": wo,
                        "mask2": mask2})
    return in_maps


def run(x, w_qkv, w_o, trace=False):
    nc = get_nc()
    in_maps = make_in_maps(x, w_qkv, w_o)
    res = run_bass_kernel_spmd(nc, in_maps, list(range(N_CORES)), trace=trace)
    outs = [res.results[i]["out"].reshape(NB, S, D) for i in range(N_CORES)]
    return np.concatenate(outs, axis=0), res


def kernel(**inputs):
    out, _ = run(inputs["x"], inputs["w_qkv"], inputs["w_o"])
    return out
